# revision 23
# baseline (speedup 1.0000x reference)
"""Causal self-attention (B=4, T=2048, C=1024, H=16) on 8 TRN2 NeuronCores.

Sharding: core c = (b, hg) with b = c//2 batch index, hg = c%2 head-group
(8 heads each).  Each core computes its batch element's attention for its 8
heads plus the partial c_proj (W_proj column-shard); the host sums the two
head-group partials per batch element.

v4: all-bf16 I/O (fp32 PSUM accumulation), stage-1 streamed per 512-token
block and emission-interleaved into the previous block's attention so the
PE fills the ACT(exp)-bound bubbles, and causal trimming of stage-2 /
exp / stage-3 work (boundary s-tiles only compute the valid column range;
stage-3 chains skip fully-masked s-tiles).

Per-core pipeline (no on-chip input transpose; host feeds xT / W tiles):
  stage 1 (bf16):  qkT[j,t] = WqkT^T-contract(xT) per 512-token block; j
                   packs head pairs as [Qa|Qb]/[Ka|Kb] 128-row chunks.
                   V[t,jv] = xT-contract(WvT), bf16 with a ones column
                   appended per head ([V_h | 1], 65 cols).
  stage 2 (bf16):  S.T[s,tq] = K lhsT vs Q rhs, two heads concurrently on
                   the PE via tile_position (0,0)/(64,0); boundary s-tiles
                   compute only cols >= q*128.
  exp (ACT):       P = exp(S.T/8) -> bf16, trimmed to the valid ranges;
                   causal mask-mul of the 128-wide diagonal block on POOL.
  stage 3 (bf16):  O[tq,65] = P^T-contract([V|1]); col 64 = softmax denom;
                   per-sub chains contract only s-tiles <= diagonal.
                   Normalize with reciprocal + per-partition scalar mul.
  transpose (PE):  y[t,j] -> yT[j,t] in 128x128 bf16 blocks.
  stage 4 (bf16):  out[t,co] = yT lhsT vs WpT rhs, accumulate over j;
                   yout written bf16 (host sums partials in fp32).
"""
import numpy as np
import ml_dtypes

import concourse.bacc as bacc
import concourse.mybir as mybir
import concourse.tile as tile
from concourse.bass_utils import run_bass_kernel_spmd

F32 = mybir.dt.float32
BF16 = mybir.dt.bfloat16

B, C, NH, HD = 4, 1024, 16, 64
HPC = 8              # heads per core
JV = HPC * HD        # 512: v-feature cols per core
KC = C // 128        # 8 contraction chunks
SCALE = 1.0 / 8.0    # 1/sqrt(HD)


def emit_body(nc, tc, dram, T):
    TT = T // 128
    TQB = T // 512
    xT, wqkT, wvT, wpT, masks, yout = (
        dram["xT"], dram["wqkT"], dram["wvT"], dram["wpT"],
        dram["masks"], dram["yout"])

    xT3 = xT.rearrange("(kc p) t -> p kc t", p=128)
    wqk3 = wqkT.rearrange("(kc p) j -> p kc j", p=128)
    wv3 = wvT.rearrange("(kc p) j -> p kc j", p=128)
    wp3 = wpT.rearrange("(jc p) co -> p jc co", p=128)

    with tc.tile_pool(name="persist", bufs=1) as pers, \
         tc.tile_pool(name="weights", bufs=1) as wpool, \
         tc.tile_pool(name="xstream", bufs=2) as xpool, \
         tc.tile_pool(name="pexp", bufs=2) as ppool, \
         tc.tile_pool(name="ypool", bufs=4) as ypool, \
         tc.tile_pool(name="ytpool", bufs=2) as ytpool, \
         tc.tile_pool(name="rcpool", bufs=8) as rcpool, \
         tc.tile_pool(name="outp", bufs=4) as outp, \
         tc.tile_pool(name="s2ps", bufs=2, space="PSUM") as s2ps, \
         tc.tile_pool(name="mmx", bufs=2, space="PSUM") as ps512, \
         tc.tile_pool(name="ps3p", bufs=2, space="PSUM") as ps3p:
        qkT_sb = pers.tile([128, 8, T], BF16)
        vext_sb = pers.tile([128, TT, HPC, 65], BF16)
        nc.vector.memset(vext_sb[:, :, :, 64:65], 1.0)
        masks_sb = pers.tile([128, 4, 2, 128], BF16)
        iden_bf = pers.tile([128, 128], BF16)

        # ---- prologue: wqk/x0 chunk pairs interleaved so the first qk
        # accumulation chains start after one chunk pair; wv/wp/masks/iden
        # trail behind stage-1 compute ----
        wqk_sb = wpool.tile([128, KC, 1024], BF16)
        wv_sb = wpool.tile([128, KC, JV], BF16)
        wp_sb = wpool.tile([128, 4, C], BF16)
        xts = {}

        def xdma(tb):
            tbs = slice(tb * 512, (tb + 1) * 512)
            xt = xpool.tile([128, KC, 512], BF16, tag="x", name=f"xt{tb}")
            xts[tb] = xt
            nc.sync.dma_start(xt[:], xT3[:, :, tbs])

        xt0 = xpool.tile([128, KC, 512], BF16, tag="x", name="xt0")
        xts[0] = xt0
        for kc in range(KC):
            nc.sync.dma_start(wqk_sb[:, kc, :], wqk3[:, kc, :])
            nc.sync.dma_start(xt0[:, kc, :], xT3[:, kc, 0:512])

        def late_dma():
            nc.sync.dma_start(wv_sb[:], wv3[:])
            nc.sync.dma_start(
                masks_sb[:], masks.rearrange("q p h f -> p q h f"))
            nc.sync.dma_start(iden_bf[:], dram["idenb"][:])
            nc.sync.dma_start(wp_sb[:], wp3[:])

        def stage1(tb, part):
            """part 0: x DMA + qk jc 0..3; 1: qk jc 4..7; 2: V."""
            tbs = slice(tb * 512, (tb + 1) * 512)
            if part == 0 and tb not in xts:
                xdma(tb)
            xt = xts[tb]
            if part in (0, 1):
                for jc in range(4 * part, 4 * part + 4):
                    ps = ps512.tile([128, 512], F32, tag="ps512")
                    for kc in range(KC):
                        nc.tensor.matmul(
                            ps[:], wqk_sb[:, kc, jc * 128:(jc + 1) * 128],
                            xt[:, kc, :],
                            start=(kc == 0), stop=(kc == KC - 1))
                    nc.vector.tensor_copy(qkT_sb[:, jc, tbs], ps[:])
            else:
                for sub in range(4):
                    tt = tb * 4 + sub
                    ps = ps512.tile([128, 512], F32, tag="ps512")
                    for kc in range(KC):
                        nc.tensor.matmul(
                            ps[:], xt[:, kc, sub * 128:(sub + 1) * 128],
                            wv_sb[:, kc, :],
                            start=(kc == 0), stop=(kc == KC - 1))
                    nc.vector.tensor_copy(
                        vext_sb[:, tt, :, 0:64],
                        ps[:].rearrange("p (h d) -> p h d", h=HPC))

        def attention(tqb, pc, y_t):
            nst = 4 * (tqb + 1)
            pab = ppool.tile([128, TT, 1024], BF16, tag="pab")
            qs, ks = 2 * pc, 2 * pc + 1
            tq0 = tqb * 512
            for st in range(nst):
                ss = slice(st * 128, (st + 1) * 128)
                q = st - 4 * tqb
                c0 = max(q, 0) * 128   # first valid tq col in this block
                psAB = s2ps.tile([128, 1024], F32, tag="s2")
                nc.tensor.matmul(
                    psAB[:, c0:512], qkT_sb[0:64, ks, ss],
                    qkT_sb[0:64, qs, tq0 + c0:tq0 + 512],
                    start=True, stop=True, tile_position=(0, 0))
                nc.tensor.matmul(
                    psAB[:, 512 + c0:1024], qkT_sb[64:128, ks, ss],
                    qkT_sb[64:128, qs, tq0 + c0:tq0 + 512],
                    start=True, stop=True, tile_position=(64, 0))
                if c0 == 0:
                    nc.scalar.activation(
                        pab[:, st, :], psAB[:],
                        mybir.ActivationFunctionType.Exp, scale=SCALE)
                else:
                    nc.scalar.activation(
                        pab[:, st].rearrange(
                            "p (h f) -> p h f", h=2)[:, :, c0:512],
                        psAB[:].rearrange(
                            "p (h f) -> p h f", h=2)[:, :, c0:512],
                        mybir.ActivationFunctionType.Exp, scale=SCALE)
                if q >= 0:  # mask-mul only the 128-wide diagonal block
                    nc.gpsimd.tensor_mul(
                        pab[:, st].rearrange(
                            "p (h f) -> p h f",
                            h=2)[:, :, q * 128:(q + 1) * 128],
                        pab[:, st].rearrange(
                            "p (h f) -> p h f",
                            h=2)[:, :, q * 128:(q + 1) * 128],
                        masks_sb[:, q])
            for hoff in (0, 1):
                h = 2 * pc + hoff
                for sub in range(4):
                    nsub = 4 * tqb + sub + 1  # s-tiles up to the diagonal
                    ps3 = ps3p.tile([128, 65], F32, tag="s3")
                    for st in range(nsub):
                        nc.tensor.matmul(
                            ps3[:],
                            pab[:, st, hoff * 512 + sub * 128:
                                hoff * 512 + (sub + 1) * 128],
                            vext_sb[:, st, h, :],
                            start=(st == 0), stop=(st == nsub - 1))
                    rc = rcpool.tile([128, 1], F32, tag="rc")
                    nc.vector.reciprocal(rc[:], ps3[:, 64:65])
                    nc.vector.tensor_scalar_mul(
                        y_t[:, sub, h * 64:(h + 1) * 64],
                        ps3[:, 0:64], rc[:])

        def posts(tqb, y_t, copy_eng=None):
            copy_eng = copy_eng or nc.vector
            yT_t = ytpool.tile([128, 4, 512], BF16, tag="yt")
            for sub in range(4):
                nc.sync.dma_start(
                    yT_t[:, :, sub * 128:(sub + 1) * 128],
                    y_t[:, sub, :], transpose=True)
            for sub in range(4):
                for nb2 in range(2):
                    ps4 = ps512.tile([128, 512], F32, tag="ps512")
                    for jc in range(4):
                        nc.tensor.matmul(
                            ps4[:],
                            yT_t[:, jc, sub * 128:(sub + 1) * 128],
                            wp_sb[:, jc, nb2 * 512:(nb2 + 1) * 512],
                            start=(jc == 0), stop=(jc == 3))
                    ot = outp.tile([128, 512], BF16, tag="ot")
                    if copy_eng is nc.scalar and nb2 == 0:
                        nc.scalar.copy(ot[:], ps4[:])
                    else:
                        nc.vector.tensor_copy(ot[:], ps4[:])
                    t0 = (tqb * 4 + sub) * 128
                    nc.sync.dma_start(
                        yout[t0:t0 + 128, nb2 * 512:(nb2 + 1) * 512], ot[:])

        # ---- software-pipelined emission ----
        # transposes ride along with each attention pc; stage4(0..2) is
        # emitted inside the LAST block's attention so its PE work fills
        # the exp-bound bubble there.
        stage1(0, 0)
        stage1(0, 1)
        late_dma()
        stage1(0, 2)
        y_ts = {}
        for tqb in range(TQB):
            y_ts[tqb] = ypool.tile([128, 4, 512], BF16, tag="y",
                                   name=f"y_t{tqb}")
            for pc in range(4):
                attention(tqb, pc, y_ts[tqb])
                if tqb + 1 < TQB and pc < 3:
                    stage1(tqb + 1, pc)
                if tqb == TQB - 1 and pc < TQB - 1:
                    posts(pc, y_ts[pc])
        posts(TQB - 1, y_ts[TQB - 1], copy_eng=nc.scalar)


def build_nc(T=2048, reps=1):
    nc = bacc.Bacc()
    dram = dict(
        xT=nc.dram_tensor("xT", [C, T], BF16, kind="ExternalInput"),
        wqkT=nc.dram_tensor("wqkT", [C, 1024], BF16, kind="ExternalInput"),
        wvT=nc.dram_tensor("wvT", [C, JV], BF16, kind="ExternalInput"),
        wpT=nc.dram_tensor("wpT", [JV, C], BF16, kind="ExternalInput"),
        masks=nc.dram_tensor("masks", [4, 128, 2, 128], BF16,
                             kind="ExternalInput"),
        idenb=nc.dram_tensor("idenb", [128, 128], BF16, kind="ExternalInput"),
        yout=nc.dram_tensor("yout", [T, C], BF16, kind="ExternalOutput"),
    )
    with tile.TileContext(nc) as tc:
        for _ in range(reps):
            emit_body(nc, tc, dram, T)
    nc.compile()
    return nc


def shard_inputs(x, W_attn, W_proj, T):
    """Full inputs -> list of 8 per-core in_maps (all bf16)."""
    x = np.asarray(x, dtype=np.float32)
    W_attn = np.asarray(W_attn, dtype=np.float32)
    W_proj = np.asarray(W_proj, dtype=np.float32)
    bf = ml_dtypes.bfloat16

    sp = np.arange(128)[:, None]
    tf = np.arange(128)[None, :]
    m1 = (tf >= sp).astype(np.float32)          # [128 s, 128 tq] lower-keep
    masks = np.broadcast_to(m1, (4, 2, 128, 128)).transpose(0, 2, 1, 3)
    masks = np.ascontiguousarray(masks).astype(bf)   # [4, 128, 2, 128]
    iden = np.eye(128, dtype=np.float32)

    in_maps = []
    for core in range(8):
        b, hg = core // 2, core % 2
        heads = [hg * HPC + i for i in range(HPC)]
        cols = []
        for pc in range(4):
            ha, hb = heads[2 * pc], heads[2 * pc + 1]
            cols += list(range(ha * 192, ha * 192 + 64))        # Q_a
            cols += list(range(hb * 192, hb * 192 + 64))        # Q_b
            cols += list(range(ha * 192 + 64, ha * 192 + 128))  # K_a
            cols += list(range(hb * 192 + 64, hb * 192 + 128))  # K_b
        vrows = [h * 192 + 128 + d for h in heads for d in range(64)]
        in_maps.append(dict(
            xT=np.ascontiguousarray(x[b, :T].T).astype(bf),
            wqkT=np.ascontiguousarray(W_attn[cols].T).astype(bf),
            wvT=np.ascontiguousarray(W_attn[vrows].T).astype(bf),
            wpT=np.ascontiguousarray(
                W_proj[:, hg * JV:(hg + 1) * JV].T).astype(bf),
            masks=masks,
            idenb=iden.astype(bf),
        ))
    return in_maps


def gather_outputs(results, T):
    out = np.empty((B, T, C), dtype=np.float32)
    for b in range(B):
        out[b] = (results[2 * b]["yout"].astype(np.float32)
                  + results[2 * b + 1]["yout"].astype(np.float32))
    return out


_NC_CACHE = {}


def run(x, W_attn, W_proj, T=2048, trace=False):
    if T not in _NC_CACHE:
        _NC_CACHE[T] = build_nc(T)
    nc = _NC_CACHE[T]
    in_maps = shard_inputs(x, W_attn, W_proj, T)
    res = run_bass_kernel_spmd(nc, in_maps, core_ids=list(range(8)), trace=trace)
    return gather_outputs(res.results, T), res


def kernel(x, W_attn, W_proj):
    out, _ = run(x, W_attn, W_proj, T=2048)
    return out
